# revision 1
# baseline (speedup 1.0000x reference)
"""MPNN layer (NNConv-style) Trainium2 Bass kernel.

Strategy: shard by destination-node range. Core c owns nodes
[c*6250, (c+1)*6250) and every edge whose dst lands there, so no
cross-core reduction is needed. The host lays each core's edge slice
out into fixed-capacity slots grouped by 128-node destination block
(sorted layout -> segment-sum becomes a one-hot matmul accumulated in
PSUM) and ships ef pre-transposed in slot order (each device holds its
edge slice of ef; node features stay replicated and are gathered on
device). bf16 feature path, f32 PSUM accumulation. Per 128-slot tile:
  x = nf[src]                     (indirect DMA gather, bf16 64B rows)
  h^T = relu(W1^T @ ef^T + b1)    (PE masked-K matmul + ACT relu)
  We  = h @ W2perm  (per-edge 32x32, o-major col layout) (PE, 2x512)
  We -> bf16 SBUF                 (ACT copy)
  msg = reduce_i(We * x_bcast)    (DVE bf16 mult + reduce)
  agg_blk  += onehot(dst)^T @ msg (PE, PSUM accumulate)
  aggX_blk += onehot(dst)^T @ x   (PE; bias-term aggregation)
per block: agg += transpose(aggX) @ B; out = agg + bias.
"""

import sys

for _p in ("/opt/trn_rl_repo",):
    if _p not in sys.path:
        sys.path.insert(0, _p)

import numpy as np

N_NODES = 50000
N_EDGES = 200000
HID = 32
ED = 16
EH = 128
NCORES = 8
NPC = N_NODES // NCORES  # 6250 nodes per core
NBLK = (NPC + 127) // 128  # 49 destination blocks per core
BLKCAP = 640  # edge-slot capacity per block (5 tiles of 128)
TPB = BLKCAP // 128
NSLOT = NBLK * BLKCAP
NTILE = NSLOT // 128
NGRP = (NTILE + 3) // 4  # ef^T ships 4 tiles per [128,128] panel

_prog_cache = {}


def _build_program(nblk=NBLK, tpb=TPB, reps=1, skip=()):
    import concourse.bacc as bacc
    import concourse.bass as bass
    import concourse.mybir as mybir
    from concourse.tile import TileContext
    from concourse.masks import make_identity

    f32 = mybir.dt.float32
    bf = mybir.dt.bfloat16
    i32 = mybir.dt.int32
    AF = mybir.ActivationFunctionType
    ALU = mybir.AluOpType
    AX = mybir.AxisListType
    ntile = nblk * tpb
    ngrp = (ntile + 3) // 4

    nc = bacc.Bacc(
        "TRN2", target_bir_lowering=False, debug=False, num_devices=NCORES
    )
    efT_d = nc.dram_tensor("efT4", [128, ngrp * 128], bf, kind="ExternalInput")
    nf_d = nc.dram_tensor("nf16", [N_NODES, HID], bf, kind="ExternalInput")
    W1_d = nc.dram_tensor("W1b", [128, 4 * EH], bf, kind="ExternalInput")
    b1_d = nc.dram_tensor("b1c", [EH, 2], f32, kind="ExternalInput")
    W2p_d = nc.dram_tensor("W2p", [EH, HID * HID], bf, kind="ExternalInput")
    Bm_d = nc.dram_tensor("Bm", [HID, HID], bf, kind="ExternalInput")
    biasr_d = nc.dram_tensor("biasr", [128, HID], f32, kind="ExternalInput")
    meta_d = nc.dram_tensor("meta", [128, ntile * 4], i32, kind="ExternalInput")
    S_d = nc.dram_tensor("Sall", [128, ntile * 128], bf, kind="ExternalInput")
    out_d = nc.dram_tensor("out", [nblk * 128, HID], f32, kind="ExternalOutput")

    with TileContext(nc) as tc:
        with (
            tc.tile_pool(name="const", bufs=1) as cp,
            tc.tile_pool(name="work", bufs=4) as wp,
            tc.tile_pool(name="gath", bufs=6) as gp,
            tc.tile_pool(name="ps_h", bufs=2, space="PSUM") as ps_h,
            tc.tile_pool(name="ps_we", bufs=2, space="PSUM") as ps_we,
            tc.tile_pool(name="ps_agg", bufs=2, space="PSUM") as ps_agg,
            tc.tile_pool(name="ps_ax", bufs=1, space="PSUM") as ps_ax,
            tc.tile_pool(name="ps_tr", bufs=1, space="PSUM") as ps_tr,
        ):
            W1_sb = cp.tile([128, 4 * EH], bf)
            nc.sync.dma_start(out=W1_sb[:], in_=W1_d[:])
            b1_sb = cp.tile([EH, 2], f32)
            nc.sync.dma_start(out=b1_sb[:], in_=b1_d[:])
            W2p_sb = cp.tile([EH, HID * HID], bf)
            nc.sync.dma_start(out=W2p_sb[:], in_=W2p_d[:])
            Bm_sb = cp.tile([HID, HID], bf)
            nc.sync.dma_start(out=Bm_sb[:], in_=Bm_d[:])
            biasr_sb = cp.tile([128, HID], f32)
            nc.sync.dma_start(out=biasr_sb[:], in_=biasr_d[:])
            meta_sb = cp.tile([128, ntile * 4], i32)
            nc.sync.dma_start(out=meta_sb[:], in_=meta_d[:])
            S_all = cp.tile([128, ntile * 128], bf)
            nc.sync.dma_start(out=S_all[:], in_=S_d[:])
            ident = cp.tile([128, 128], bf)
            make_identity(nc, ident[:])

            agg = None
            aggX = None
            for rep in range(reps):
                for g in range(ngrp):
                    tlist = [t for t in range(4 * g, 4 * g + 4) if t < ntile]
                    efT4 = gp.tile([128, 128], bf, tag="efT4")
                    nc.sync.dma_start(
                        out=efT4[:], in_=efT_d[:, g * 128 : (g + 1) * 128]
                    )
                    for c, t in enumerate(tlist):
                        j = t % tpb
                        b = t // tpb
                        x_t = gp.tile([128, HID], bf, tag="x")
                        if "gather" in skip:
                            nc.sync.dma_start(
                                out=x_t[:], in_=nf_d[t * 128 : (t + 1) * 128, :]
                            )
                        else:
                            nc.gpsimd.indirect_dma_start(
                                out=x_t[:],
                                out_offset=None,
                                in_=nf_d[:],
                                in_offset=bass.IndirectOffsetOnAxis(
                                    ap=meta_sb[:, t * 4 + 1 : t * 4 + 2], axis=0
                                ),
                            )
                        hT_ps = ps_h.tile([EH, 128], f32, tag="h")
                        nc.tensor.matmul(
                            out=hT_ps[:],
                            lhsT=W1_sb[:, c * EH : (c + 1) * EH],
                            rhs=efT4[:],
                            start=True, stop=True,
                        )
                        h_sb = wp.tile([EH, 128], bf, tag="hsb")
                        nc.scalar.activation(
                            out=h_sb[:], in_=hT_ps[:], func=AF.Relu,
                            bias=b1_sb[:, 0:1], scale=1.0,
                        )
                        prod = wp.tile([128, HID, HID], bf, tag="prod")
                        for hh in range(2):
                            We_ps = ps_we.tile([128, 512], f32, tag="we")
                            nc.tensor.matmul(
                                out=We_ps[:],
                                lhsT=h_sb[:],
                                rhs=W2p_sb[:, hh * 512 : (hh + 1) * 512],
                                start=True, stop=True,
                            )
                            We_sb = wp.tile([128, 512], bf, tag="wesb")
                            nc.scalar.copy(out=We_sb[:], in_=We_ps[:])
                            xb = x_t[:, None, :].to_broadcast([128, 16, HID])
                            nc.vector.tensor_tensor(
                                out=prod[:, hh * 16 : (hh + 1) * 16, :],
                                in0=We_sb[:].rearrange(
                                    "p (o i) -> p o i", i=HID
                                ),
                                in1=xb,
                                op=ALU.mult,
                            )
                        t1 = wp.tile([128, HID, 16], bf, tag="t1")
                        nc.vector.tensor_tensor(
                            out=t1[:], in0=prod[:, :, 0:16],
                            in1=prod[:, :, 16:32], op=ALU.add,
                        )
                        t2 = wp.tile([128, HID, 8], bf, tag="t2")
                        nc.vector.tensor_tensor(
                            out=t2[:], in0=t1[:, :, 0:8],
                            in1=t1[:, :, 8:16], op=ALU.add,
                        )
                        t3 = wp.tile([128, HID, 4], bf, tag="t3")
                        nc.vector.tensor_tensor(
                            out=t3[:], in0=t2[:, :, 0:4],
                            in1=t2[:, :, 4:8], op=ALU.add,
                        )
                        t4 = wp.tile([128, HID, 2], bf, tag="t4")
                        nc.vector.tensor_tensor(
                            out=t4[:], in0=t3[:, :, 0:2],
                            in1=t3[:, :, 2:4], op=ALU.add,
                        )
                        msg2 = wp.tile([128, HID], bf, tag="msg2")
                        nc.vector.tensor_tensor(
                            out=msg2[:],
                            in0=t4[:, :, 0:1].rearrange("p o one -> p (o one)"),
                            in1=t4[:, :, 1:2].rearrange("p o one -> p (o one)"),
                            op=ALU.add,
                        )
                        if j == 0:
                            agg = ps_agg.tile([128, HID], f32, tag="agg")
                            aggX = ps_ax.tile([128, HID], f32, tag="aggX")
                        nc.tensor.matmul(
                            out=agg[:],
                            lhsT=S_all[:, t * 128 : (t + 1) * 128],
                            rhs=msg2[:],
                            start=(j == 0), stop=False,
                        )
                        nc.tensor.matmul(
                            out=aggX[:],
                            lhsT=S_all[:, t * 128 : (t + 1) * 128],
                            rhs=x_t[:],
                            start=(j == 0), stop=(j == tpb - 1),
                        )
                        if j == tpb - 1:
                            # bias term: agg += aggX @ B  (transpose aggX
                            # on PE, then one K=32 matmul into same bank)
                            aggX_sb = wp.tile([128, HID], bf, tag="axsb")
                            nc.scalar.copy(out=aggX_sb[:], in_=aggX[:])
                            axT_ps = ps_tr.tile([HID, 128], bf, tag="axT")
                            nc.tensor.transpose(
                                out=axT_ps[:], in_=aggX_sb[:],
                                identity=ident[:],
                            )
                            axT_sb = wp.tile([HID, 128], bf, tag="axT_sb")
                            nc.scalar.copy(out=axT_sb[:], in_=axT_ps[:])
                            nc.tensor.matmul(
                                out=agg[:], lhsT=axT_sb[:], rhs=Bm_sb[:],
                                start=False, stop=True,
                            )
                            ob = wp.tile([128, HID], f32, tag="ob")
                            nc.vector.tensor_tensor(
                                out=ob[:], in0=agg[:], in1=biasr_sb[:],
                                op=ALU.add,
                            )
                            nc.sync.dma_start(
                                out=out_d[b * 128 : (b + 1) * 128, :],
                                in_=ob[:],
                            )
    nc.compile()
    return nc


def _host_layout(edge_src, edge_dst):
    """Slot layout per core + overflow edge list (rarely non-empty)."""
    metas, eidxs, overflow = [], [], []
    core = edge_dst // NPC
    for c in range(NCORES):
        sel = np.nonzero(core == c)[0].astype(np.int64)
        ld = edge_dst[sel].astype(np.int64) - c * NPC
        blk = ld >> 7
        order = np.argsort(blk, kind="stable")
        se, sblk, sld = sel[order], blk[order], ld[order]
        counts = np.bincount(sblk, minlength=NBLK)
        starts = np.concatenate(([0], np.cumsum(counts)[:-1]))
        pos = np.arange(len(se)) - starts[sblk]
        keep = pos < BLKCAP
        slot = sblk[keep] * BLKCAP + pos[keep]
        meta = np.zeros((NSLOT, 4), dtype=np.int32)
        meta[:, 2] = -1
        eidx = np.full(NSLOT, -1, dtype=np.int64)
        eidx[slot] = se[keep]
        meta[slot, 0] = se[keep].astype(np.int32)
        meta[slot, 1] = edge_src[se[keep]].astype(np.int32)
        meta[slot, 2] = (sld[keep] & 127).astype(np.int32)
        meta_r = np.ascontiguousarray(
            meta.reshape(NTILE, 128, 4).transpose(1, 0, 2).reshape(128, -1)
        )
        metas.append(meta_r)
        eidxs.append(eidx)
        overflow.extend(se[~keep].tolist())
    return metas, eidxs, overflow


def _make_in_maps(nf, ef, edge_src, edge_dst, W1, b1, W2, b2, bias):
    import ml_dtypes

    bf = ml_dtypes.bfloat16
    metas, eidxs, overflow = _host_layout(edge_src, edge_dst)
    ef_bf = ef.astype(bf)
    W2p = np.ascontiguousarray(
        W2.reshape(EH, HID, HID).transpose(0, 2, 1).reshape(EH, HID * HID)
    ).astype(bf)
    W1r = np.zeros((128, 4 * EH), dtype=bf)
    for c in range(4):
        W1r[c * 32 : c * 32 + ED, c * EH : (c + 1) * EH] = W1.astype(bf)
    common = {
        "nf16": nf.astype(bf),
        "W1b": W1r,
        "b1c": np.ascontiguousarray(np.tile(b1.reshape(EH, 1), (1, 2))),
        "W2p": W2p,
        "Bm": np.ascontiguousarray(b2.reshape(HID, HID)).astype(bf),
        "biasr": np.ascontiguousarray(np.tile(bias[None, :], (128, 1))),
    }
    in_maps = []
    for c in range(NCORES):
        dst_cols = metas[c].reshape(128, NTILE, 4)[:, :, 2]  # [128, NTILE]
        S_nt = np.zeros((128, NTILE, 128), dtype=bf)
        pp, tt = np.nonzero(dst_cols >= 0)
        S_nt[pp, tt, dst_cols[pp, tt]] = 1
        S_all = np.ascontiguousarray(S_nt.reshape(128, NTILE * 128))
        # ef slice in slot order, transposed, packed 4 tiles per panel
        # (tile 4g+q at rows 32q..32q+16 of panel g)
        e_slots = np.zeros((NGRP * 4 * 128, ED), dtype=bf)
        eidx = eidxs[c]
        valid = eidx >= 0
        sl = e_slots[:NSLOT]
        sl[valid] = ef_bf[eidx[valid]]
        et = e_slots.reshape(NGRP, 4, 128, ED).transpose(0, 1, 3, 2)
        efT4 = np.zeros((128, NGRP, 128), dtype=bf)
        for q in range(4):
            efT4[32 * q : 32 * q + ED] = et[:, q].transpose(1, 0, 2)
        efT4 = np.ascontiguousarray(efT4.reshape(128, NGRP * 128))
        in_maps.append(
            {**common, "efT4": efT4, "meta": metas[c], "Sall": S_all}
        )
    return in_maps, overflow


def kernel(nf, ef, edge_src, edge_dst, W1, b1, W2, b2, bias):
    from concourse.bass_utils import run_bass_kernel_spmd

    nf = np.asarray(nf, dtype=np.float32)
    ef = np.asarray(ef, dtype=np.float32)
    edge_src = np.asarray(edge_src, dtype=np.int32)
    edge_dst = np.asarray(edge_dst, dtype=np.int32)
    W1 = np.asarray(W1, dtype=np.float32)
    b1 = np.asarray(b1, dtype=np.float32)
    W2 = np.asarray(W2, dtype=np.float32)
    b2 = np.asarray(b2, dtype=np.float32)
    bias = np.asarray(bias, dtype=np.float32)

    if "prog" not in _prog_cache:
        _prog_cache["prog"] = _build_program()
    nc = _prog_cache["prog"]

    in_maps, overflow = _make_in_maps(
        nf, ef, edge_src, edge_dst, W1, b1, W2, b2, bias
    )

    res = run_bass_kernel_spmd(nc, in_maps, core_ids=list(range(NCORES)))
    out = np.concatenate(
        [res.results[c]["out"][:NPC] for c in range(NCORES)], axis=0
    )

    if overflow:  # capacity spill: finish the stragglers on host
        e = np.asarray(overflow, dtype=np.int64)
        h = np.maximum(ef[e] @ W1 + b1, 0.0)
        We = (h @ W2 + b2).reshape(-1, HID, HID)
        msg = np.einsum("ei,eio->eo", nf[edge_src[e]], We)
        np.add.at(out, edge_dst[e], msg)

    return np.ascontiguousarray(out, dtype=np.float32)



# revision 3
# speedup vs baseline: 1.3402x; 1.3402x over previous
"""MPNN layer (NNConv-style) Trainium2 Bass kernel, v2.

Strategy: shard by destination node. Core c owns nodes [c*6250, (c+1)*6250).
Host packs that core's edges (sorted by dst) into NG=52 groups, each
covering <=128 consecutive nodes and <=512 edges (4 tiles of 128 slots);
avg degree is exactly 4 so both caps bind together (~97% slot utilization).
Host pre-gathers source features and pre-transposes ef into slot order.

Per 128-edge tile on device:
  h^T  = relu(W1^T @ ef^T + b1)      PE (masked-K 4-tile panel) + ACT relu
  We   = h^T' @ W2p (o-major)        PE, [128,1024] f32 PSUM
  prod = We * x_bcast  -> bf16 SBUF  split 3 ways to balance engines:
           cols 0:256   ACT copy -> DVE mult (2x bf16)
           cols 256:672 ACT copy -> Pool mult
           cols 672:1024 DVE mult direct from PSUM
  Z   += onehot(dst)^T @ prod        PE, per-group PSUM accumulation
  xt  += x^T-aggregate via matmul    PE ([32,128] PSUM)
Per group: agg = reduce_i(Z) on DVE -> DMA out; xt -> SBUF -> DMA out.
Host adds the b2 term (aggX @ b2r), output bias, and rare spilled edges.
"""

import sys

for _p in ("/opt/trn_rl_repo",):
    if _p not in sys.path:
        sys.path.insert(0, _p)

import numpy as np

N_NODES = 50000
N_EDGES = 200000
HID = 32
ED = 16
EH = 128
NCORES = 8
NPC = N_NODES // NCORES  # 6250 nodes per core
NODE_CAP = 128
EDGE_CAP = 512
NG = 52  # groups per core (seed-0 data needs <=51; spill covers the rest)
NT = NG * 4  # 208 tiles per core
NCH = NG // 4  # 13 DMA chunks of 4 groups / 16 tiles

# prod column split (o-groups of 32): ACT-copy+DVE, ACT-copy+Pool, DVE-direct
A_COLS = 256
P_COLS = 416
ACP = A_COLS + P_COLS  # copied to SBUF by ACT
D_COLS = 1024 - ACP

_prog_cache = {}


def _build_program():
    import concourse.bacc as bacc
    import concourse.mybir as mybir
    from concourse.tile import TileContext

    f32 = mybir.dt.float32
    bf = mybir.dt.bfloat16
    AF = mybir.ActivationFunctionType
    ALU = mybir.AluOpType
    AX = mybir.AxisListType

    nc = bacc.Bacc(
        "TRN2", target_bir_lowering=False, debug=False, num_devices=NCORES
    )
    S_d = nc.dram_tensor("Sall", [128, NT * 128], bf, kind="ExternalInput")
    x_d = nc.dram_tensor("xsl", [128, NT * 32], bf, kind="ExternalInput")
    efT_d = nc.dram_tensor("efT", [128, NG * 128], bf, kind="ExternalInput")
    W1_d = nc.dram_tensor("W1b", [128, 4 * EH], bf, kind="ExternalInput")
    b1_d = nc.dram_tensor("b1c", [EH, 2], f32, kind="ExternalInput")
    W2_d = nc.dram_tensor("W2p", [EH, HID * HID], bf, kind="ExternalInput")
    agg_d = nc.dram_tensor("aggout", [NG * 128, HID], f32, kind="ExternalOutput")
    xt_d = nc.dram_tensor("xtout", [NG * HID, 128], f32, kind="ExternalOutput")

    with TileContext(nc) as tc:
        with (
            tc.tile_pool(name="const", bufs=1) as cp,
            tc.tile_pool(name="sch", bufs=3) as sp,
            tc.tile_pool(name="ech", bufs=3) as ep,
            tc.tile_pool(name="xch", bufs=3) as xp,
            tc.tile_pool(name="hsb", bufs=2) as hp,
            tc.tile_pool(name="wsb", bufs=3) as wp,
            tc.tile_pool(name="prod", bufs=8) as pp,
            tc.tile_pool(name="aggs", bufs=2) as ap_,
            tc.tile_pool(name="xts", bufs=2) as xtp,
            tc.tile_pool(name="ps_h", bufs=1, space="PSUM") as ps_h,
            tc.tile_pool(name="ps_we", bufs=2, space="PSUM") as ps_we,
            tc.tile_pool(name="ps_z", bufs=1, space="PSUM") as ps_z,
            tc.tile_pool(name="ps_xt", bufs=1, space="PSUM") as ps_xt,
        ):
            W1_sb = cp.tile([128, 4 * EH], bf)
            nc.sync.dma_start(out=W1_sb[:], in_=W1_d[:])
            b1_sb = cp.tile([EH, 2], f32)
            nc.sync.dma_start(out=b1_sb[:], in_=b1_d[:])
            W2_sb = cp.tile([EH, HID * HID], bf)
            nc.sync.dma_start(out=W2_sb[:], in_=W2_d[:])

            s_ch = {}
            e_ch = {}
            x_ch = {}

            def load_chunk(chi):
                s_ch[chi] = sp.tile([128, 16 * 128], bf, tag="S", name=f"sch{chi}")
                nc.sync.dma_start(
                    out=s_ch[chi][:],
                    in_=S_d[:, chi * 2048 : (chi + 1) * 2048],
                )
                e_ch[chi] = ep.tile([128, 4 * 128], bf, tag="ef", name=f"ech{chi}")
                nc.sync.dma_start(
                    out=e_ch[chi][:],
                    in_=efT_d[:, chi * 512 : (chi + 1) * 512],
                )
                x_ch[chi] = xp.tile([128, 16 * 32], bf, tag="x", name=f"xch{chi}")
                nc.sync.dma_start(
                    out=x_ch[chi][:],
                    in_=x_d[:, chi * 512 : (chi + 1) * 512],
                )

            load_chunk(0)

            prods = {}
            xts = {}
            for g in range(NG + 1):
                # -- group-g edge stage prologue: chunk prefetch + h --
                if g < NG:
                    chi = g // 4
                    gg = g % 4
                    if gg == 0 and chi + 1 < NCH:
                        load_chunk(chi + 1)
                    h_ps = ps_h.tile([EH, 512], f32, tag="h")
                    for c in range(4):
                        nc.tensor.matmul(
                            out=h_ps[:, c * 128 : (c + 1) * 128],
                            lhsT=W1_sb[:, c * EH : (c + 1) * EH],
                            rhs=e_ch[chi][:, gg * 128 : (gg + 1) * 128],
                            start=True, stop=True,
                        )
                    h_sb = hp.tile([EH, 512], bf, tag="h")
                    nc.scalar.activation(
                        out=h_sb[:], in_=h_ps[:], func=AF.Relu,
                        bias=b1_sb[:, 0:1], scale=1.0,
                    )
                # -- aggregate stage for group g-1 --
                if g >= 1:
                    pg = g - 1
                    z = ps_z.tile([128, 1024], f32, tag="z")
                    pchi = pg // 4
                    for c in range(4):
                        t = 4 * pg + c
                        tloc = t - pchi * 16
                        S_sl = s_ch[pchi][:, tloc * 128 : (tloc + 1) * 128]
                        pr = prods.pop(t)
                        nc.tensor.matmul(
                            out=z[:, 0:512], lhsT=S_sl, rhs=pr[:, 0:512],
                            start=(c == 0), stop=(c == 3),
                        )
                        nc.tensor.matmul(
                            out=z[:, 512:1024], lhsT=S_sl, rhs=pr[:, 512:1024],
                            start=(c == 0), stop=(c == 3),
                        )
                    agg_sb = ap_.tile([128, HID], f32, tag="agg")
                    nc.vector.tensor_reduce(
                        out=agg_sb[:],
                        in_=z[:].rearrange("p (o i) -> p o i", i=HID),
                        axis=AX.X, op=ALU.add,
                    )
                    nc.sync.dma_start(
                        out=agg_d[pg * 128 : (pg + 1) * 128, :], in_=agg_sb[:]
                    )
                    xt_sb = xtp.tile([HID, 128], f32, tag="xt")
                    nc.scalar.copy(out=xt_sb[:], in_=xts.pop(pg)[:])
                    nc.sync.dma_start(
                        out=xt_d[pg * HID : (pg + 1) * HID, :], in_=xt_sb[:]
                    )
                # -- group-g edge stage main: We, prod, xt --
                if g < NG:
                    for c in range(4):
                        t = 4 * g + c
                        tloc = t - chi * 16
                        we = ps_we.tile([128, 1024], f32, tag="we")
                        nc.tensor.matmul(
                            out=we[:, 0:512],
                            lhsT=h_sb[:, c * 128 : (c + 1) * 128],
                            rhs=W2_sb[:, 0:512],
                            start=True, stop=True,
                        )
                        nc.tensor.matmul(
                            out=we[:, 512:1024],
                            lhsT=h_sb[:, c * 128 : (c + 1) * 128],
                            rhs=W2_sb[:, 512:1024],
                            start=True, stop=True,
                        )
                        x_t = x_ch[chi][:, tloc * 32 : (tloc + 1) * 32]
                        S_sl = s_ch[chi][:, tloc * 128 : (tloc + 1) * 128]
                        if c == 0:
                            xts[g] = ps_xt.tile([HID, 128], f32, tag="xt", name=f"xt{g}")
                        nc.tensor.matmul(
                            out=xts[g][:], lhsT=x_t, rhs=S_sl,
                            start=(c == 0), stop=(c == 3),
                        )
                        wsb = wp.tile([128, ACP], bf, tag="we")
                        nc.scalar.copy(out=wsb[:], in_=we[:, 0:ACP])
                        pr = pp.tile([128, 1024], bf, tag="prod")
                        prods[t] = pr
                        xa = x_t[:, None, :].to_broadcast(
                            [128, A_COLS // HID, HID]
                        )
                        nc.vector.tensor_tensor(
                            out=pr[:, 0:A_COLS].rearrange(
                                "p (o i) -> p o i", i=HID
                            ),
                            in0=wsb[:, 0:A_COLS].rearrange(
                                "p (o i) -> p o i", i=HID
                            ),
                            in1=xa, op=ALU.mult,
                        )
                        xg = x_t[:, None, :].to_broadcast(
                            [128, P_COLS // HID, HID]
                        )
                        nc.gpsimd.tensor_tensor(
                            out=pr[:, A_COLS:ACP].rearrange(
                                "p (o i) -> p o i", i=HID
                            ),
                            in0=wsb[:, A_COLS:ACP].rearrange(
                                "p (o i) -> p o i", i=HID
                            ),
                            in1=xg, op=ALU.mult,
                        )
                        xd = x_t[:, None, :].to_broadcast(
                            [128, D_COLS // HID, HID]
                        )
                        nc.vector.tensor_tensor(
                            out=pr[:, ACP:1024].rearrange(
                                "p (o i) -> p o i", i=HID
                            ),
                            in0=we[:, ACP:1024].rearrange(
                                "p (o i) -> p o i", i=HID
                            ),
                            in1=xd, op=ALU.mult,
                        )
    nc.compile()
    return nc


def _layout_core(edge_src, edge_dst, ef_bf, nf_bf, c):
    """Group packing + slot layout for core c. Returns device arrays,
    reassembly map, and spilled edge ids."""
    import ml_dtypes

    bfl = ml_dtypes.bfloat16
    sel = np.nonzero((edge_dst // NPC) == c)[0]
    dl_all = edge_dst[sel].astype(np.int64) - c * NPC
    order = np.argsort(dl_all, kind="stable")
    se = sel[order]
    dl = dl_all[order]
    deg = np.bincount(dl, minlength=NPC)
    run_start = np.concatenate(([0], np.cumsum(deg)[:-1]))
    pos = np.arange(len(se)) - run_start[dl]
    used = pos < EDGE_CAP
    spill = list(se[~used])
    se_u = se[used]
    dl_u = dl[used]
    deg_u = np.minimum(deg, EDGE_CAP)

    g_n0, g_ncnt, g_e0, g_ecnt = [], [], [], []
    cn = ce = 0
    n0 = e0 = cum = 0
    for n in range(NPC):
        d = int(deg_u[n])
        if cn >= NODE_CAP or ce + d > EDGE_CAP:
            g_n0.append(n0)
            g_ncnt.append(cn)
            g_e0.append(e0)
            g_ecnt.append(ce)
            n0, e0, cn, ce = n, cum, 0, 0
        cn += 1
        ce += d
        cum += d
    g_n0.append(n0)
    g_ncnt.append(cn)
    g_e0.append(e0)
    g_ecnt.append(ce)

    if len(g_n0) > NG:  # capacity exceeded: host-compute the tail
        cut_e = g_e0[NG]
        spill.extend(se_u[cut_e:].tolist())
        se_u, dl_u = se_u[:cut_e], dl_u[:cut_e]
        g_n0, g_ncnt = g_n0[:NG], g_ncnt[:NG]
        g_e0, g_ecnt = g_e0[:NG], g_ecnt[:NG]

    G = len(g_n0)
    ncov = len(se_u)
    e0s = np.asarray(g_e0, dtype=np.int64)
    n0s = np.asarray(g_n0, dtype=np.int64)
    eidx = np.arange(ncov)
    g_of = np.searchsorted(e0s, eidx, side="right") - 1
    slot = eidx - e0s[g_of] + 512 * g_of
    tile = slot >> 7
    row = slot & 127
    rank = dl_u - n0s[g_of]

    S = np.zeros((128, NT, 128), dtype=bfl)
    S[row, tile, rank] = 1
    xsl = np.zeros((128, NT, HID), dtype=bfl)
    xsl[row, tile] = nf_bf[edge_src[se_u]]
    efsl = np.zeros((NT * 128, ED), dtype=bfl)
    efsl[slot] = ef_bf[se_u]
    eft = np.zeros((128, NG, 128), dtype=bfl)
    efr = efsl.reshape(NG, 4, 128, ED)
    for cpos in range(4):
        eft[32 * cpos : 32 * cpos + ED] = efr[:, cpos].transpose(2, 0, 1)

    dev = {
        "Sall": np.ascontiguousarray(S.reshape(128, NT * 128)),
        "xsl": np.ascontiguousarray(xsl.reshape(128, NT * HID)),
        "efT": np.ascontiguousarray(eft.reshape(128, NG * 128)),
    }
    remap = (n0s, np.asarray(g_ncnt, dtype=np.int64), G)
    return dev, remap, spill


def _make_in_maps(nf, ef, edge_src, edge_dst, W1, b1, W2, b2, bias):
    import ml_dtypes

    bfl = ml_dtypes.bfloat16
    nf_bf = nf.astype(bfl)
    ef_bf = ef.astype(bfl)
    W2p = np.ascontiguousarray(
        W2.reshape(EH, HID, HID).transpose(0, 2, 1).reshape(EH, HID * HID)
    ).astype(bfl)
    W1r = np.zeros((128, 4 * EH), dtype=bfl)
    for c in range(4):
        W1r[c * 32 : c * 32 + ED, c * EH : (c + 1) * EH] = W1.astype(bfl)
    common = {
        "W1b": W1r,
        "b1c": np.ascontiguousarray(np.tile(b1.reshape(EH, 1), (1, 2))),
        "W2p": W2p,
    }
    in_maps, remaps, spill = [], [], []
    for c in range(NCORES):
        dev, remap, sp = _layout_core(edge_src, edge_dst, ef_bf, nf_bf, c)
        in_maps.append({**common, **dev})
        remaps.append(remap)
        spill.extend(sp)
    return in_maps, remaps, spill


def kernel(nf, ef, edge_src, edge_dst, W1, b1, W2, b2, bias):
    from concourse.bass_utils import run_bass_kernel_spmd

    nf = np.asarray(nf, dtype=np.float32)
    ef = np.asarray(ef, dtype=np.float32)
    edge_src = np.asarray(edge_src, dtype=np.int32)
    edge_dst = np.asarray(edge_dst, dtype=np.int32)
    W1 = np.asarray(W1, dtype=np.float32)
    b1 = np.asarray(b1, dtype=np.float32)
    W2 = np.asarray(W2, dtype=np.float32)
    b2 = np.asarray(b2, dtype=np.float32)
    bias = np.asarray(bias, dtype=np.float32)

    if "prog" not in _prog_cache:
        _prog_cache["prog"] = _build_program()
    nc = _prog_cache["prog"]

    in_maps, remaps, spill = _make_in_maps(
        nf, ef, edge_src, edge_dst, W1, b1, W2, b2, bias
    )
    res = run_bass_kernel_spmd(nc, in_maps, core_ids=list(range(NCORES)))

    b2r = b2.reshape(HID, HID)
    out = np.tile(bias[None, :], (N_NODES, 1)).astype(np.float32)
    for c in range(NCORES):
        n0s, ncnts, G = remaps[c]
        agg = np.asarray(res.results[c]["aggout"], dtype=np.float32)
        xt = np.asarray(res.results[c]["xtout"], dtype=np.float32)
        aggX = xt.reshape(NG, HID, 128).transpose(0, 2, 1).reshape(NG * 128, HID)
        tot = agg + aggX @ b2r
        node_idx = np.concatenate(
            [np.arange(n0s[g], n0s[g] + ncnts[g]) for g in range(G)]
        )
        rows = np.concatenate(
            [g * 128 + np.arange(ncnts[g]) for g in range(G)]
        )
        out[c * NPC + node_idx] += tot[rows]

    if spill:  # capacity spill: finish the stragglers on host
        e = np.asarray(spill, dtype=np.int64)
        h = np.maximum(ef[e] @ W1 + b1, 0.0)
        We = (h @ W2 + b2).reshape(-1, HID, HID)
        msg = np.einsum("ei,eio->eo", nf[edge_src[e]], We)
        np.add.at(out, edge_dst[e], msg)

    return np.ascontiguousarray(out, dtype=np.float32)


# revision 4
# speedup vs baseline: 1.3928x; 1.0392x over previous
"""MPNN layer (NNConv-style) Trainium2 Bass kernel, v2.

Strategy: shard by destination node. Core c owns nodes [c*6250, (c+1)*6250).
Host packs that core's edges (sorted by dst) into NG=52 groups, each
covering <=128 consecutive nodes and <=512 edges (4 tiles of 128 slots);
avg degree is exactly 4 so both caps bind together (~97% slot utilization).
Host pre-gathers source features and pre-transposes ef into slot order.

Per 128-edge tile on device:
  h^T  = relu(W1^T @ ef^T + b1)      PE (masked-K 4-tile panel) + ACT relu
  We   = h^T' @ W2p (o-major)        PE, [128,1024] f32 PSUM
  prod = We * x_bcast  -> bf16 SBUF  split 3 ways to balance engines:
           cols 0:256   ACT copy -> DVE mult (2x bf16)
           cols 256:672 ACT copy -> Pool mult
           cols 672:1024 DVE mult direct from PSUM
  Z   += onehot(dst)^T @ prod        PE, per-group PSUM accumulation
  xt  += x^T-aggregate via matmul    PE ([32,128] PSUM)
Per group: agg = reduce_i(Z) on DVE -> DMA out; xt -> SBUF -> DMA out.
Host adds the b2 term (aggX @ b2r), output bias, and rare spilled edges.
"""

import sys

for _p in ("/opt/trn_rl_repo",):
    if _p not in sys.path:
        sys.path.insert(0, _p)

import numpy as np

N_NODES = 50000
N_EDGES = 200000
HID = 32
ED = 16
EH = 128
NCORES = 8
NPC = N_NODES // NCORES  # 6250 nodes per core
NODE_CAP = 128
EDGE_CAP = 512
NG = 52  # groups per core (seed-0 data needs <=51; spill covers the rest)
NT = NG * 4  # 208 tiles per core
NCH = NG // 4  # 13 DMA chunks of 4 groups / 16 tiles

# prod column split (o-groups of 32): ACT-copy+DVE, ACT-copy+Pool, DVE-direct
A_COLS = 256
P_COLS = 416
ACP = A_COLS + P_COLS  # copied to SBUF by ACT
D_COLS = 1024 - ACP

_prog_cache = {}


def _build_program():
    import concourse.bacc as bacc
    import concourse.mybir as mybir
    from concourse.tile import TileContext

    f32 = mybir.dt.float32
    bf = mybir.dt.bfloat16
    AF = mybir.ActivationFunctionType
    ALU = mybir.AluOpType
    AX = mybir.AxisListType

    nc = bacc.Bacc(
        "TRN2", target_bir_lowering=False, debug=False, num_devices=NCORES
    )
    S_d = nc.dram_tensor("Sall", [128, NT * 128], bf, kind="ExternalInput")
    x_d = nc.dram_tensor("xsl", [128, NT * 32], bf, kind="ExternalInput")
    efT_d = nc.dram_tensor("efT", [128, NG * 128], bf, kind="ExternalInput")
    W1_d = nc.dram_tensor("W1b", [128, 4 * EH], bf, kind="ExternalInput")
    b1_d = nc.dram_tensor("b1c", [EH, 2], f32, kind="ExternalInput")
    W2_d = nc.dram_tensor("W2p", [EH, HID * HID], bf, kind="ExternalInput")
    agg_d = nc.dram_tensor("aggout", [NG * 128, HID], f32, kind="ExternalOutput")
    xt_d = nc.dram_tensor("xtout", [NG * HID, 128], f32, kind="ExternalOutput")

    with TileContext(nc) as tc:
        with (
            tc.tile_pool(name="const", bufs=1) as cp,
            tc.tile_pool(name="sch", bufs=3) as sp,
            tc.tile_pool(name="ech", bufs=3) as ep,
            tc.tile_pool(name="xch", bufs=3) as xp,
            tc.tile_pool(name="hsb", bufs=2) as hp,
            tc.tile_pool(name="wsb", bufs=3) as wp,
            tc.tile_pool(name="prod", bufs=8) as pp,
            tc.tile_pool(name="aggs", bufs=2) as ap_,
            tc.tile_pool(name="xts", bufs=2) as xtp,
            tc.tile_pool(name="ps_h", bufs=1, space="PSUM") as ps_h,
            tc.tile_pool(name="ps_we", bufs=2, space="PSUM") as ps_we,
            tc.tile_pool(name="ps_z", bufs=1, space="PSUM") as ps_z,
            tc.tile_pool(name="ps_xt", bufs=1, space="PSUM") as ps_xt,
        ):
            W1_sb = cp.tile([128, 4 * EH], bf)
            nc.sync.dma_start(out=W1_sb[:], in_=W1_d[:])
            b1_sb = cp.tile([EH, 2], f32)
            nc.sync.dma_start(out=b1_sb[:], in_=b1_d[:])
            W2_sb = cp.tile([EH, HID * HID], bf)
            nc.sync.dma_start(out=W2_sb[:], in_=W2_d[:])

            s_ch = {}
            e_ch = {}
            x_ch = {}

            def load_chunk(chi):
                s_ch[chi] = sp.tile([128, 16 * 128], bf, tag="S", name=f"sch{chi}")
                nc.sync.dma_start(
                    out=s_ch[chi][:],
                    in_=S_d[:, chi * 2048 : (chi + 1) * 2048],
                )
                e_ch[chi] = ep.tile([128, 4 * 128], bf, tag="ef", name=f"ech{chi}")
                nc.sync.dma_start(
                    out=e_ch[chi][:],
                    in_=efT_d[:, chi * 512 : (chi + 1) * 512],
                )
                x_ch[chi] = xp.tile([128, 16 * 32], bf, tag="x", name=f"xch{chi}")
                nc.sync.dma_start(
                    out=x_ch[chi][:],
                    in_=x_d[:, chi * 512 : (chi + 1) * 512],
                )

            load_chunk(0)

            prods = {}
            xts = {}
            z_tiles = {}

            def emit_tile(g, c):
                """We matmuls + prod (3-way split) + xt for tile c of g."""
                chi = g // 4
                t = 4 * g + c
                tloc = t - chi * 16
                we = ps_we.tile([128, 1024], f32, tag="we", name=f"we{t}")
                nc.tensor.matmul(
                    out=we[:, 0:512],
                    lhsT=h_sbs[g][:, c * 128 : (c + 1) * 128],
                    rhs=W2_sb[:, 0:512],
                    start=True, stop=True,
                )
                nc.tensor.matmul(
                    out=we[:, 512:1024],
                    lhsT=h_sbs[g][:, c * 128 : (c + 1) * 128],
                    rhs=W2_sb[:, 512:1024],
                    start=True, stop=True,
                )
                x_t = x_ch[chi][:, tloc * 32 : (tloc + 1) * 32]
                S_sl = s_ch[chi][:, tloc * 128 : (tloc + 1) * 128]
                if c == 0:
                    xts[g] = ps_xt.tile([HID, 128], f32, tag="xt", name=f"xt{g}")
                nc.tensor.matmul(
                    out=xts[g][:], lhsT=x_t, rhs=S_sl,
                    start=(c == 0), stop=(c == 3),
                )
                pr = pp.tile([128, 1024], bf, tag="prod", name=f"pr{t}")
                prods[t] = pr
                # direct-PSUM DVE mult first: no ACT-copy dependency, frees
                # the we bank for the next-but-one We matmul sooner
                xd = x_t[:, None, :].to_broadcast([128, D_COLS // HID, HID])
                nc.vector.tensor_tensor(
                    out=pr[:, ACP:1024].rearrange("p (o i) -> p o i", i=HID),
                    in0=we[:, ACP:1024].rearrange("p (o i) -> p o i", i=HID),
                    in1=xd, op=ALU.mult,
                )
                wsb = wp.tile([128, ACP], bf, tag="we", name=f"wsb{t}")
                nc.scalar.copy(out=wsb[:], in_=we[:, 0:ACP])
                xa = x_t[:, None, :].to_broadcast([128, A_COLS // HID, HID])
                nc.vector.tensor_tensor(
                    out=pr[:, 0:A_COLS].rearrange("p (o i) -> p o i", i=HID),
                    in0=wsb[:, 0:A_COLS].rearrange("p (o i) -> p o i", i=HID),
                    in1=xa, op=ALU.mult,
                )
                xg = x_t[:, None, :].to_broadcast([128, P_COLS // HID, HID])
                nc.gpsimd.tensor_tensor(
                    out=pr[:, A_COLS:ACP].rearrange("p (o i) -> p o i", i=HID),
                    in0=wsb[:, A_COLS:ACP].rearrange("p (o i) -> p o i", i=HID),
                    in1=xg, op=ALU.mult,
                )

            def emit_zpair(pg, c01):
                """Two Z accumulation tiles (c01, c01+1) for group pg."""
                pchi = pg // 4
                if c01 == 0:
                    z_tiles[pg] = ps_z.tile(
                        [128, 1024], f32, tag="z", name=f"z{pg}"
                    )
                z = z_tiles[pg]
                for c in (c01, c01 + 1):
                    t = 4 * pg + c
                    tloc = t - pchi * 16
                    S_sl = s_ch[pchi][:, tloc * 128 : (tloc + 1) * 128]
                    pr = prods.pop(t)
                    nc.tensor.matmul(
                        out=z[:, 0:512], lhsT=S_sl, rhs=pr[:, 0:512],
                        start=(c == 0), stop=(c == 3),
                    )
                    nc.tensor.matmul(
                        out=z[:, 512:1024], lhsT=S_sl, rhs=pr[:, 512:1024],
                        start=(c == 0), stop=(c == 3),
                    )

            def emit_epilogue(pg):
                """Reduce Z -> agg, evacuate xt; DMA both out."""
                z = z_tiles.pop(pg)
                agg_sb = ap_.tile([128, HID], f32, tag="agg", name=f"agg{pg}")
                nc.vector.tensor_reduce(
                    out=agg_sb[:],
                    in_=z[:].rearrange("p (o i) -> p o i", i=HID),
                    axis=AX.X, op=ALU.add,
                )
                nc.sync.dma_start(
                    out=agg_d[pg * 128 : (pg + 1) * 128, :], in_=agg_sb[:]
                )
                xt_sb = xtp.tile([HID, 128], f32, tag="xt", name=f"xts{pg}")
                nc.scalar.copy(out=xt_sb[:], in_=xts.pop(pg)[:])
                nc.sync.dma_start(
                    out=xt_d[pg * HID : (pg + 1) * HID, :], in_=xt_sb[:]
                )

            h_sbs = {}
            for g in range(NG + 1):
                # chunk prefetch + h matmuls + relu for group g
                if g < NG:
                    chi = g // 4
                    gg = g % 4
                    if gg == 0 and chi + 1 < NCH:
                        load_chunk(chi + 1)
                    h_ps = ps_h.tile([EH, 512], f32, tag="h")
                    for c in range(4):
                        nc.tensor.matmul(
                            out=h_ps[:, c * 128 : (c + 1) * 128],
                            lhsT=W1_sb[:, c * EH : (c + 1) * EH],
                            rhs=e_ch[chi][:, gg * 128 : (gg + 1) * 128],
                            start=True, stop=True,
                        )
                    h_sbs[g] = hp.tile([EH, 512], bf, tag="h", name=f"h{g}")
                    nc.scalar.activation(
                        out=h_sbs[g][:], in_=h_ps[:], func=AF.Relu,
                        bias=b1_sb[:, 0:1], scale=1.0,
                    )
                # interleave: We/prod tiles of g with Z accumulation of g-1
                # (Z-reduce of g-1 lands mid-group so its WAR on the single
                # z bank resolves before group g's Z matmuls begin)
                if g < NG:
                    emit_tile(g, 0)
                if g >= 1:
                    emit_zpair(g - 1, 0)
                if g < NG:
                    emit_tile(g, 1)
                if g >= 1:
                    emit_zpair(g - 1, 2)
                    emit_epilogue(g - 1)
                if g < NG:
                    emit_tile(g, 2)
                    emit_tile(g, 3)
    nc.compile()
    return nc


def _layout_core(edge_src, edge_dst, ef_bf, nf_bf, c):
    """Group packing + slot layout for core c. Returns device arrays,
    reassembly map, and spilled edge ids."""
    import ml_dtypes

    bfl = ml_dtypes.bfloat16
    sel = np.nonzero((edge_dst // NPC) == c)[0]
    dl_all = edge_dst[sel].astype(np.int64) - c * NPC
    order = np.argsort(dl_all, kind="stable")
    se = sel[order]
    dl = dl_all[order]
    deg = np.bincount(dl, minlength=NPC)
    run_start = np.concatenate(([0], np.cumsum(deg)[:-1]))
    pos = np.arange(len(se)) - run_start[dl]
    used = pos < EDGE_CAP
    spill = list(se[~used])
    se_u = se[used]
    dl_u = dl[used]
    deg_u = np.minimum(deg, EDGE_CAP)

    g_n0, g_ncnt, g_e0, g_ecnt = [], [], [], []
    cn = ce = 0
    n0 = e0 = cum = 0
    for n in range(NPC):
        d = int(deg_u[n])
        if cn >= NODE_CAP or ce + d > EDGE_CAP:
            g_n0.append(n0)
            g_ncnt.append(cn)
            g_e0.append(e0)
            g_ecnt.append(ce)
            n0, e0, cn, ce = n, cum, 0, 0
        cn += 1
        ce += d
        cum += d
    g_n0.append(n0)
    g_ncnt.append(cn)
    g_e0.append(e0)
    g_ecnt.append(ce)

    if len(g_n0) > NG:  # capacity exceeded: host-compute the tail
        cut_e = g_e0[NG]
        spill.extend(se_u[cut_e:].tolist())
        se_u, dl_u = se_u[:cut_e], dl_u[:cut_e]
        g_n0, g_ncnt = g_n0[:NG], g_ncnt[:NG]
        g_e0, g_ecnt = g_e0[:NG], g_ecnt[:NG]

    G = len(g_n0)
    ncov = len(se_u)
    e0s = np.asarray(g_e0, dtype=np.int64)
    n0s = np.asarray(g_n0, dtype=np.int64)
    eidx = np.arange(ncov)
    g_of = np.searchsorted(e0s, eidx, side="right") - 1
    slot = eidx - e0s[g_of] + 512 * g_of
    tile = slot >> 7
    row = slot & 127
    rank = dl_u - n0s[g_of]

    S = np.zeros((128, NT, 128), dtype=bfl)
    S[row, tile, rank] = 1
    xsl = np.zeros((128, NT, HID), dtype=bfl)
    xsl[row, tile] = nf_bf[edge_src[se_u]]
    efsl = np.zeros((NT * 128, ED), dtype=bfl)
    efsl[slot] = ef_bf[se_u]
    eft = np.zeros((128, NG, 128), dtype=bfl)
    efr = efsl.reshape(NG, 4, 128, ED)
    for cpos in range(4):
        eft[32 * cpos : 32 * cpos + ED] = efr[:, cpos].transpose(2, 0, 1)

    dev = {
        "Sall": np.ascontiguousarray(S.reshape(128, NT * 128)),
        "xsl": np.ascontiguousarray(xsl.reshape(128, NT * HID)),
        "efT": np.ascontiguousarray(eft.reshape(128, NG * 128)),
    }
    remap = (n0s, np.asarray(g_ncnt, dtype=np.int64), G)
    return dev, remap, spill


def _make_in_maps(nf, ef, edge_src, edge_dst, W1, b1, W2, b2, bias):
    import ml_dtypes

    bfl = ml_dtypes.bfloat16
    nf_bf = nf.astype(bfl)
    ef_bf = ef.astype(bfl)
    W2p = np.ascontiguousarray(
        W2.reshape(EH, HID, HID).transpose(0, 2, 1).reshape(EH, HID * HID)
    ).astype(bfl)
    W1r = np.zeros((128, 4 * EH), dtype=bfl)
    for c in range(4):
        W1r[c * 32 : c * 32 + ED, c * EH : (c + 1) * EH] = W1.astype(bfl)
    common = {
        "W1b": W1r,
        "b1c": np.ascontiguousarray(np.tile(b1.reshape(EH, 1), (1, 2))),
        "W2p": W2p,
    }
    in_maps, remaps, spill = [], [], []
    for c in range(NCORES):
        dev, remap, sp = _layout_core(edge_src, edge_dst, ef_bf, nf_bf, c)
        in_maps.append({**common, **dev})
        remaps.append(remap)
        spill.extend(sp)
    return in_maps, remaps, spill


def kernel(nf, ef, edge_src, edge_dst, W1, b1, W2, b2, bias):
    from concourse.bass_utils import run_bass_kernel_spmd

    nf = np.asarray(nf, dtype=np.float32)
    ef = np.asarray(ef, dtype=np.float32)
    edge_src = np.asarray(edge_src, dtype=np.int32)
    edge_dst = np.asarray(edge_dst, dtype=np.int32)
    W1 = np.asarray(W1, dtype=np.float32)
    b1 = np.asarray(b1, dtype=np.float32)
    W2 = np.asarray(W2, dtype=np.float32)
    b2 = np.asarray(b2, dtype=np.float32)
    bias = np.asarray(bias, dtype=np.float32)

    if "prog" not in _prog_cache:
        _prog_cache["prog"] = _build_program()
    nc = _prog_cache["prog"]

    in_maps, remaps, spill = _make_in_maps(
        nf, ef, edge_src, edge_dst, W1, b1, W2, b2, bias
    )
    res = run_bass_kernel_spmd(nc, in_maps, core_ids=list(range(NCORES)))

    b2r = b2.reshape(HID, HID)
    out = np.tile(bias[None, :], (N_NODES, 1)).astype(np.float32)
    for c in range(NCORES):
        n0s, ncnts, G = remaps[c]
        agg = np.asarray(res.results[c]["aggout"], dtype=np.float32)
        xt = np.asarray(res.results[c]["xtout"], dtype=np.float32)
        aggX = xt.reshape(NG, HID, 128).transpose(0, 2, 1).reshape(NG * 128, HID)
        tot = agg + aggX @ b2r
        node_idx = np.concatenate(
            [np.arange(n0s[g], n0s[g] + ncnts[g]) for g in range(G)]
        )
        rows = np.concatenate(
            [g * 128 + np.arange(ncnts[g]) for g in range(G)]
        )
        out[c * NPC + node_idx] += tot[rows]

    if spill:  # capacity spill: finish the stragglers on host
        e = np.asarray(spill, dtype=np.int64)
        h = np.maximum(ef[e] @ W1 + b1, 0.0)
        We = (h @ W2 + b2).reshape(-1, HID, HID)
        msg = np.einsum("ei,eio->eo", nf[edge_src[e]], We)
        np.add.at(out, edge_dst[e], msg)

    return np.ascontiguousarray(out, dtype=np.float32)
